# revision 9
# baseline (speedup 1.0000x reference)
"""Brute-force N^2 neighborlist (PBC, cutoff) on 8 Trainium2 NeuronCores.

Strategy
--------
The graded inputs use the static upper-triangle pair list (i<j) of
N=6144 atoms, so instead of streaming 151 MB of indices and gathering,
each core computes a dense "wrapped diagonal band": for its 768 rows i,
all neighbor offsets t in [1, 3072], pair (i, (i+t) mod N). Every
unordered pair {i,j} appears exactly once among rows min-side (plus the
3072 antipodal t=3072 duplicates). The host maps the band back onto the
triu pair order with one vectorized gather (negating the 3 displacement
components where the band stored the reversed pair).

All cores run one identical SPMD program; per-core differences live only
in input data: a skewed copy of (-positions) so that SBUF row k holds
-pos[(768c + k + v) mod N] at column v (making pos_i - pos_j a single
per-partition tensor_scalar add), and per-row tables for the PBC wrap.

PBC wrap without mod/floor (neither exists on this toolchain): for a
fixed row i, delta = pos_i - pos_j lies in (pos_i-10, pos_i], which
spans only ONE wrap boundary, so wrap(delta) = delta + A_i + g where
A_i in {0,10} and g = -10*[ -pos_j >= tau_i ] with per-row constants
  pos_i >= 5:  A=0,  tau = 5 - pos_i   (subtract 10 when delta >= 5)
  pos_i <  5:  A=10, tau = -5 - pos_i  (net +10 when delta < -5)
Then d2 = sum of squares (Square on ACT, adds on GPSIMD), mask =
[d2 <= 0.25], outputs (wrap * mask per component, sqrt(d2 * mask)).

Falls back to a pure-numpy reference path if the inputs are not the
standard triu/eye(3)*10 setup.
"""

import numpy as np

N = 6144
NCORES = 8
RPC = N // NCORES          # rows per core = 768
NBLK = RPC // 128          # 6 row-blocks of 128 partitions
T = N // 2                 # neighbor offsets t in [1, 3072]
CHUNK = 512
NCH = T // CHUNK           # 6 chunks per block
SKEW_W = 128 * (NBLK - 1) + T + 1   # 3713 columns used; tile width below
SKEW_TILE_W = 3840
SKEW_SRC_W = SKEW_TILE_W + 128   # overlapping skewed read needs k+v range
BOX_L = 10.0
CUTOFF = 0.5

_cache = {}


def _apply_tilepatch():
    """Workarounds for this walrus build: <=1 sem-wait per normal
    instruction (2 for EventSemaphore), and no waits on the kernel-tail
    Drain. Tile attaches unbounded waits; spill the excess onto
    EventSemaphore NOPs."""
    if _cache.get("patched"):
        return
    import bass_rust
    import concourse.mybir as mybir
    import concourse.tile as tile_mod
    from concourse.tile import TileContext
    from concourse.tile_clock_wait import TileClockWait as _Real
    from concourse.vector_clock import ScopedClock

    counter = [0]

    def _cap(inst):
        return 2 if isinstance(inst, mybir.InstEventSemaphore) else 1

    def _spill(ordered):
        for bb_name in list(ordered.keys()):
            insts = ordered[bb_name]
            out, changed = [], False
            for inst in insts:
                si = inst.sync_info
                cap = _cap(inst)
                if si is not None and len(si.on_wait) > cap:
                    changed = True
                    waits = list(si.on_wait)
                    keep, spill = waits[-cap:], waits[:-cap]
                    for i in range(0, len(spill), 2):
                        counter[0] += 1
                        nop = mybir.InstEventSemaphore(
                            name=f"wspill_{counter[0]}", ins=[], outs=[]
                        )
                        nop.engine = inst.engine
                        nop.sync_info = bass_rust.SyncInfo(
                            on_wait=spill[i : i + 2], on_update=[]
                        )
                        out.append(nop)
                    inst.sync_info = bass_rust.SyncInfo(
                        on_wait=keep, on_update=list(si.on_update)
                    )
                out.append(inst)
            if changed:
                ordered[bb_name] = out

    class _Fixed:
        def __init__(self, tc, ordered, **kw):
            self._inner = _Real(tc, ordered, **kw)
            self._ordered = ordered

        def assign_waits(self, bb_name):
            self._inner.assign_waits(bb_name)
            _spill(self._ordered)

        def __getattr__(self, name):
            return getattr(self._inner, name)

    def _drain_and_barrier(self, tick_clock, wait_clock):
        nc = self.nc
        probe = mybir.InstEventSemaphore(
            name=f"tilefix_probe_{nc.next_id()}", ins=[], outs=[]
        )
        probe.engine = mybir.EngineType.SP
        wait_clock.add_sem_waits(
            probe, ScopedClock({None: tick_clock.global_clock})
        )
        si = probe.sync_info
        waits = list(si.on_wait) if si is not None else []
        assert self.sems is not None
        byname = {h.name: h for h in self.sems.allocated().values()}
        for i in range(0, len(waits), 2):
            w0 = waits[i]
            inst = nc.sync.wait_ge(byname[w0.ant_name], w0.wait_value)
            if i + 1 < len(waits):
                w1 = waits[i + 1]
                inst.wait_op(byname[w1.ant_name], w1.wait_value, "sem-ge")
        nc.sync.drain()
        nc.all_engine_barrier()
        popped = nc._tile_sem_poison_stack.pop()
        assert popped is self._sem_poison
        nc.clear_and_free_semaphores(list(self.sems.allocated().values()))
        nc.all_engine_barrier()

    tile_mod.TileClockWait = _Fixed
    TileContext._drain_and_barrier = _drain_and_barrier
    _cache["patched"] = True


def build_program():
    """Build (once) the SPMD Bass program shared by all 8 cores."""
    if "nc" in _cache:
        return _cache["nc"]
    _apply_tilepatch()
    import concourse.bass as bass
    import concourse.mybir as mybir
    from concourse.ap import AP
    from concourse.tile import TileContext

    f32 = mybir.dt.float32
    OP = mybir.AluOpType
    AF = mybir.ActivationFunctionType

    nc = bass.Bass()
    skew = nc.declare_dram_parameter("skew", [3 * SKEW_SRC_W], f32, isOutput=False)
    mtab = nc.declare_dram_parameter("mtab", [3, 128, NBLK], f32, isOutput=False)
    ttab = nc.declare_dram_parameter("ttab", [3, 128, NBLK], f32, isOutput=False)
    atab = nc.declare_dram_parameter("atab", [3, 128, NBLK], f32, isOutput=False)
    outs = [
        nc.declare_dram_parameter(nm, [RPC, T], f32, isOutput=True)
        for nm in ("ox", "oy", "oz", "od")
    ]

    with TileContext(nc) as tc:
        with (
            tc.tile_pool(name="res", bufs=1) as res,
            tc.tile_pool(name="work", bufs=2) as work,
        ):
            S = []
            for c in range(3):
                st = res.tile([128, SKEW_TILE_W], f32, tag=f"S{c}")
                nc.sync.dma_start(
                    out=st[:, :],
                    in_=AP(skew, c * SKEW_SRC_W, [[1, 128], [1, SKEW_TILE_W]]),
                )
                S.append(st)
            tabs = {}
            for nm, dram in (("m", mtab), ("t", ttab), ("a", atab)):
                for c in range(3):
                    tt = res.tile([128, NBLK], f32, tag=f"{nm}{c}")
                    nc.sync.dma_start(out=tt[:, :], in_=dram[c])
                    tabs[nm, c] = tt

            for m in range(NBLK):
                for q in range(NCH):
                    off = 128 * m + CHUNK * q + 1
                    w3, sq3, o3 = [], [], []
                    for c in range(3):
                        src = S[c][:, off : off + CHUNK]
                        delta = work.tile([128, CHUNK], f32, tag=f"delta{c}")
                        nc.vector.tensor_scalar(
                            out=delta[:, :], in0=src,
                            scalar1=tabs["m", c][:, m : m + 1], scalar2=None,
                            op0=OP.add,
                        )
                        g = work.tile([128, CHUNK], f32, tag=f"g{c}")
                        nc.vector.tensor_scalar(
                            out=g[:, :], in0=src,
                            scalar1=tabs["t", c][:, m : m + 1], scalar2=-10.0,
                            op0=OP.is_ge, op1=OP.mult,
                        )
                        w = work.tile([128, CHUNK], f32, tag=f"w{c}")
                        nc.vector.scalar_tensor_tensor(
                            out=w[:, :], in0=delta[:, :],
                            scalar=tabs["a", c][:, m : m + 1], in1=g[:, :],
                            op0=OP.add, op1=OP.add,
                        )
                        w3.append(w)
                        sq = work.tile([128, CHUNK], f32, tag=f"sq{c}")
                        nc.scalar.activation(sq[:, :], w[:, :], AF.Square)
                        sq3.append(sq)
                    d2 = work.tile([128, CHUNK], f32, tag="d2")
                    nc.gpsimd.tensor_tensor(
                        out=d2[:, :], in0=sq3[0][:, :], in1=sq3[1][:, :], op=OP.add
                    )
                    nc.gpsimd.tensor_tensor(
                        out=d2[:, :], in0=d2[:, :], in1=sq3[2][:, :], op=OP.add
                    )
                    mk = work.tile([128, CHUNK], f32, tag="mk")
                    nc.vector.tensor_scalar(
                        out=mk[:, :], in0=d2[:, :], scalar1=CUTOFF * CUTOFF,
                        scalar2=None, op0=OP.is_le,
                    )
                    for c in range(3):
                        o = work.tile([128, CHUNK], f32, tag=f"o{c}")
                        nc.vector.tensor_tensor(
                            out=o[:, :], in0=w3[c][:, :], in1=mk[:, :], op=OP.mult
                        )
                        o3.append(o)
                    dm = work.tile([128, CHUNK], f32, tag="dm")
                    nc.vector.scalar_tensor_tensor(
                        out=dm[:, :], in0=d2[:, :], scalar=CUTOFF * CUTOFF,
                        in1=d2[:, :], op0=OP.is_le, op1=OP.mult,
                    )
                    od = work.tile([128, CHUNK], f32, tag="od")
                    nc.scalar.activation(od[:, :], dm[:, :], AF.Sqrt)
                    for buf, dram in zip(o3 + [od], outs):
                        nc.sync.dma_start(
                            out=dram[128 * m : 128 * (m + 1),
                                     CHUNK * q : CHUNK * (q + 1)],
                            in_=buf[:, :],
                        )
    _cache["nc"] = nc
    return nc


def make_in_maps(positions):
    """Per-core input data for the SPMD program."""
    pos = np.ascontiguousarray(positions, dtype=np.float32)  # [N, 3]
    in_maps = []
    for c in range(NCORES):
        base = RPC * c
        idx = (base + np.arange(SKEW_SRC_W)) % N
        skew = -pos[idx].T.astype(np.float32)        # [3, SKEW_TILE_W]
        rows = base + (np.arange(NBLK)[None, :] * 128
                       + np.arange(128)[:, None])     # [128, NBLK]
        pi = pos[rows]                                # [128, NBLK, 3]
        m = np.moveaxis(pi, 2, 0).astype(np.float32)  # [3, 128, NBLK]
        hi = m >= 5.0
        tau = np.where(hi, 5.0 - m, -5.0 - m).astype(np.float32)
        a = np.where(hi, 0.0, 10.0).astype(np.float32)
        in_maps.append({
            "skew": np.ascontiguousarray(skew.reshape(-1)),
            "mtab": np.ascontiguousarray(m),
            "ttab": np.ascontiguousarray(tau),
            "atab": np.ascontiguousarray(a),
        })
    return in_maps


def _make_runner():
    """Compile the SPMD program to a cached jitted callable (one XLA/NEFF
    compile per process). Mirrors bass2jax.run_bass_via_pjrt's multi-core
    path, but keeps the jitted function so repeat calls don't recompile."""
    if "runner" in _cache:
        return _cache["runner"]
    import jax
    import numpy as _np
    import concourse.mybir as mybir
    from jax.sharding import Mesh, PartitionSpec
    from jax.experimental.shard_map import shard_map
    from concourse import bass2jax
    from concourse.bass2jax import _bass_exec_p, install_neuronx_cc_hook

    nc = build_program()
    install_neuronx_cc_hook()
    partition_name = (
        nc.partition_id_tensor.name if nc.partition_id_tensor else None
    )
    in_names, out_names, out_avals = [], [], []
    for alloc in nc.m.functions[0].allocations:
        if not isinstance(alloc, mybir.MemoryLocationSet):
            continue
        name = alloc.memorylocations[0].name
        if alloc.kind == "ExternalInput":
            if name != partition_name:
                in_names.append(name)
        elif alloc.kind == "ExternalOutput":
            shape = tuple(alloc.tensor_shape)
            dtype = mybir.dt.np(alloc.dtype)
            out_names.append(name)
            out_avals.append(jax.core.ShapedArray(shape, dtype))
    n_params = len(in_names)

    # Every output element is written by the kernel, so the pre-zeroed
    # donated output operands run_bass_via_pjrt uses are unnecessary;
    # dropping them keeps per-call H2D traffic to the ~350 KB of real
    # inputs, which lets wall-clock timing of the jitted call approximate
    # device exec time (outputs stay device-resident until fetched).
    body_in_names = list(in_names)
    if partition_name is not None:
        body_in_names.append(partition_name)

    def _body(*args):
        operands = list(args)
        if partition_name is not None:
            operands.append(bass2jax.partition_id_tensor())
        outs = _bass_exec_p.bind(
            *operands,
            out_avals=tuple(out_avals),
            in_names=tuple(body_in_names),
            out_names=tuple(out_names),
            lowering_input_output_aliases=(),
            sim_require_finite=True,
            sim_require_nnan=True,
            nc=nc,
        )
        return tuple(outs)

    devices = jax.devices()[:NCORES]
    mesh = Mesh(_np.asarray(devices), ("core",))
    sharded = jax.jit(
        shard_map(
            _body, mesh=mesh,
            in_specs=(PartitionSpec("core"),) * n_params,
            out_specs=(PartitionSpec("core"),) * len(out_names),
            check_rep=False,
        ),
        keep_unused=True,
    )

    def runner(in_maps, fetch=True):
        concat_in = [
            np.concatenate([np.asarray(in_maps[c][nm]) for c in range(NCORES)],
                           axis=0)
            for nm in in_names
        ]
        out_arrs = sharded(*concat_in)
        if not fetch:
            for a in out_arrs:
                a.block_until_ready()
            return out_arrs
        return [
            {
                nm: np.asarray(out_arrs[k]).reshape(
                    (NCORES,) + out_avals[k].shape)[c]
                for k, nm in enumerate(out_names)
            }
            for c in range(NCORES)
        ]

    _cache["runner"] = runner
    return runner


def run_on_hw(positions, trace=False, fetch=True):
    """Run the SPMD program; returns results (list of per-core dicts) or a
    BassKernelResults when trace=True (separate compile, for profiling)."""
    in_maps = make_in_maps(positions)
    if trace:
        from concourse.bass_utils import run_bass_kernel_spmd

        nc = build_program()
        return run_bass_kernel_spmd(nc, in_maps, list(range(NCORES)), trace=True)
    return _make_runner()(in_maps, fetch=fetch)


def _triu_ij():
    if "ij" not in _cache:
        i, j = np.triu_indices(N, k=1)
        _cache["ij"] = (i.astype(np.int64), j.astype(np.int64))
    return _cache["ij"]


def _is_standard(positions, box_vectors, pair_i, pair_j):
    if positions.shape != (N, 3) or box_vectors.shape != (3, 3):
        return False
    if not np.allclose(box_vectors, np.eye(3, dtype=np.float32) * BOX_L):
        return False
    i, j = _triu_ij()
    if pair_i.shape != i.shape or pair_j.shape != j.shape:
        return False
    return bool(np.array_equal(pair_i, i) and np.array_equal(pair_j, j))


def _numpy_fallback(positions, box_vectors, pair_i, pair_j):
    pos = positions.astype(np.float32)
    r = pos[pair_i] - pos[pair_j]
    L = np.diagonal(box_vectors).astype(np.float32)
    half = (L * np.float32(0.5)).astype(np.float32)
    r = (np.remainder(r + half, L) - half).astype(np.float32)
    d = np.sqrt(np.sum(r * r, axis=1, keepdims=True)).astype(np.float32)
    out = np.where(d <= np.float32(CUTOFF),
                   np.concatenate([r, d], axis=-1), np.float32(0.0))
    return out.astype(np.float32)


def assemble(results, pair_i=None, pair_j=None):
    """Map the 8 per-core diagonal bands onto the triu pair order."""
    planes = [
        np.concatenate([results[c][nm] for c in range(NCORES)], axis=0)
        for nm in ("ox", "oy", "oz", "od")
    ]  # each [N, T]
    if pair_i is None:
        i, j = _triu_ij()
    else:
        i, j = pair_i.astype(np.int64), pair_j.astype(np.int64)
    delta = j - i
    near = delta <= T
    row = np.where(near, i, j)
    tcol = np.where(near, delta, N - delta) - 1
    flat = row * T + tcol
    sign = np.where(near, np.float32(1.0), np.float32(-1.0))
    P = i.shape[0]
    out = np.empty((P, 4), dtype=np.float32)
    for k in range(3):
        out[:, k] = planes[k].reshape(-1)[flat] * sign
    out[:, 3] = planes[3].reshape(-1)[flat]
    return out


def kernel(positions, box_vectors, pair_i, pair_j):
    positions = np.asarray(positions, dtype=np.float32)
    box_vectors = np.asarray(box_vectors, dtype=np.float32)
    pair_i = np.asarray(pair_i)
    pair_j = np.asarray(pair_j)
    if not _is_standard(positions, box_vectors, pair_i, pair_j):
        return _numpy_fallback(positions, box_vectors, pair_i, pair_j)
    res = run_on_hw(positions)
    return assemble(res)
